# revision 18
# baseline (speedup 1.0000x reference)
"""Trainium2 Bass kernel for dynamic adaptive-pooling depthwise conv.

Problem: x [16,128,192,192] f32. Per-sample selector head (global mean ->
MLP -> softmax over K=2) mixes a bank of K depthwise 3x3 kernels; then a
per-(sample,channel) 3x3 depthwise conv + bias.

Strategy (8 NeuronCores, data-parallel over batch, 2 samples/core),
single-read design: x is read from HBM exactly once.
  - per 24-row tile: HWDGE f32 load (contiguous) -> GPSIMD casts into a
    RESIDENT zero-padded bf16 SBUF tile, fusing per-channel sums via
    accum_out (mean comes for free).
  - selector head entirely as tiny PE matmuls (no cross-partition ops),
    with PE-warmup dummy matmuls woven through its dependency chain.
  - conv: 7 of 9 taps as PSUM-accumulated diag-stationary matmuls
    (tap-major groups to amortize LDWEIGHTS); the two center-column taps
    (one carrying bias) on ACT (bf16, 4B-aligned for 2x mode); DVE sums
    the two tap maps (bf16 2x) and merges psum+taps into contiguous f32
    staging; stores ride the second HWDGE ring (nc.scalar).

kernel(**inputs) takes FULL inputs, shards batch over 8 cores, returns
FULL output. Self-contained: hardcodes all shapes.
"""
import numpy as np

B, C, H, W = 16, 128, 192, 192
NCORES = 8
BC = B // NCORES          # samples per core
RS = 196                  # padded row stride (192 img + 4 pad cols, even)
LEAD = 2                  # leading pad elems (keeps tap offsets 4B-aligned)
R = 24                    # output rows per conv tile
NT = H // R               # conv tiles per sample (8)
NPAIR = R // 2            # psum row-pairs per conv tile (12)
XT_FLAT = LEAD + (R + 2) * RS + 2
CH = 13                   # max rows per f32 load chunk
GRP = 6                   # row-pairs per psum group (tap-major)
PE_TAPS = [0, 2, 3, 5, 6, 7, 8]   # taps 1 and 4 (center column) on ACT

_cache = {}


def _build():
    from concourse import bacc, mybir
    from concourse.tile import TileContext

    f32 = mybir.dt.float32
    bf16 = mybir.dt.bfloat16
    AF = mybir.ActivationFunctionType
    ALU = mybir.AluOpType
    AX = mybir.AxisListType

    nc = bacc.Bacc()
    x_ext = nc.declare_dram_parameter("x", [BC, C, H, W], f32, isOutput=False)
    out_ext = nc.declare_dram_parameter("out", [BC, C, H, W], f32, isOutput=True)
    w1T_ext = nc.declare_dram_parameter("w1T", [C, 32], f32, isOutput=False)
    b1_ext = nc.declare_dram_parameter("b1c", [32, 1], f32, isOutput=False)
    w2T_ext = nc.declare_dram_parameter("w2T", [32, 2], f32, isOutput=False)
    b2_ext = nc.declare_dram_parameter("b2c", [2, 1], f32, isOutput=False)
    ones2_ext = nc.declare_dram_parameter("ones2", [2, C], f32, isOutput=False)
    bankT_ext = nc.declare_dram_parameter("bankT", [2, 9 * C], f32, isOutput=False)
    bias_ext = nc.declare_dram_parameter("biasc", [C, 1], f32, isOutput=False)
    ident_ext = nc.declare_dram_parameter("ident", [C, C], f32, isOutput=False)

    with TileContext(nc) as tc:
        with (
            tc.tile_pool(name="consts", bufs=1) as consts,
            tc.tile_pool(name="stg", bufs=3) as stgp,
            tc.tile_pool(name="xt", bufs=NT + 1) as xcp,
            tc.tile_pool(name="stat", bufs=2) as statp,
            tc.tile_pool(name="sel", bufs=2) as selp,
            tc.tile_pool(name="diag", bufs=2) as diagp,
            tc.tile_pool(name="tap", bufs=3) as tapp,
            tc.tile_pool(name="outp", bufs=2) as outp,
            tc.tile_pool(name="psc", bufs=6, space="PSUM") as psc,
            tc.tile_pool(name="pss", bufs=2, space="PSUM") as pss,
        ):
            def cload(shape, ext, tag):
                t = consts.tile(shape, f32, tag=tag)
                nc.sync.dma_start(out=t, in_=ext[:, :])
                return t
            w1T_sb = cload([C, 32], w1T_ext, "c_w1T")
            b1_sb = cload([32, 1], b1_ext, "c_b1")
            w2T_sb = cload([32, 2], w2T_ext, "c_w2T")
            b2_sb = cload([2, 1], b2_ext, "c_b2")
            ones2_sb = cload([2, C], ones2_ext, "c_ones2")
            bankT_sb = cload([2, 9 * C], bankT_ext, "c_bankT")
            bias_sb = cload([C, 1], bias_ext, "c_bias")
            ident_sb = cload([C, C], ident_ext, "c_ident")
            identbf = consts.tile([C, C], bf16, tag="c_identbf")
            nc.vector.tensor_copy(identbf, ident_sb)

            def load_tile(b, ti, partials):
                """Load+cast one 24-row tile; fused per-channel sums."""
                r0 = ti * R
                xt = xcp.tile([C, XT_FLAT], bf16, tag="xt")
                xt3 = xt[:, LEAD:LEAD + (R + 2) * RS].rearrange(
                    "p (r c) -> p r c", c=RS)
                nc.gpsimd.memset(xt[:, 0:LEAD], 0.0)
                nc.gpsimd.memset(xt[:, XT_FLAT - 2:XT_FLAT], 0.0)
                nc.gpsimd.memset(xt3[:, :, 192:196], 0.0)
                if ti == 0:
                    nc.gpsimd.memset(xt3[:, 0:1, 0:192], 0.0)
                if ti == NT - 1:
                    nc.gpsimd.memset(xt3[:, R + 1:R + 2, 0:192], 0.0)
                HR = R // 2   # interior rows per chunk (12)
                for half in range(2):
                    if half == 0:
                        qa = r0 - 1 if ti > 0 else r0
                        qb = r0 + HR
                        ha = 1 if ti > 0 else 0
                    else:
                        qa = r0 + HR
                        qb = r0 + 2 * HR + 1 if ti < NT - 1 else r0 + 2 * HR
                        ha = 1 if ti < NT - 1 else 0
                    nrow = qb - qa
                    stg = stgp.tile([C, CH * W], f32, tag="stg")
                    nc.sync.dma_start(out=stg[:, :nrow * W],
                                      in_=x_ext[b][:, qa:qb, :])
                    s3 = stg[:, :nrow * W].rearrange("p (r c) -> p r c", c=W)
                    if half == 0:
                        if ha:
                            nc.gpsimd.tensor_copy(xt3[:, 0:1, 0:192],
                                                  s3[:, 0:1, :])
                        nc.gpsimd.tensor_copy(
                            xt3[:, 1:1 + HR, 0:192], s3[:, ha:ha + HR, :])
                    else:
                        nc.gpsimd.tensor_copy(
                            xt3[:, 1 + HR:1 + 2 * HR, 0:192], s3[:, 0:HR, :])
                        if ha:
                            nc.gpsimd.tensor_copy(
                                xt3[:, R + 1:R + 2, 0:192], s3[:, HR:HR + 1, :])
                # per-channel sum of the interior (pads are zero): in-place
                # identity copy on ACT with free-dim accumulator
                xin = xt[:, LEAD + RS:LEAD + (R + 1) * RS]
                nc.scalar.activation(xin, xin, AF.Copy,
                                     accum_out=partials[:, ti:ti + 1])
                return xt

            def selector(b, partials, warm_src):
                def dummy(n):
                    for _ in range(n):
                        dps = pss.tile([C, 512], f32, tag="selps")
                        nc.tensor.matmul(dps, identbf[:, :],
                                         warm_src[:, LEAD:LEAD + 512],
                                         start=True, stop=True)
                pooled = statp.tile([C, 1], f32, tag="pooled")
                nc.vector.reduce_sum(pooled, partials, axis=AX.X)
                hA = pss.tile([32, 1], f32, tag="selps")
                nc.tensor.matmul(hA, w1T_sb[:, :], pooled[:, :], start=True, stop=True)
                if warm_src is not None:
                    dummy(4)
                hs = selp.tile([32, 1], f32, tag="hs")
                nc.scalar.activation(hs, hA, AF.Relu, bias=b1_sb[:, :])
                lB = pss.tile([2, 1], f32, tag="selps")
                nc.tensor.matmul(lB, w2T_sb[:, :], hs[:, :], start=True, stop=True)
                if warm_src is not None:
                    dummy(4)
                es = selp.tile([2, 1], f32, tag="es")
                nc.scalar.activation(es, lB, AF.Exp, bias=b2_sb[:, :])
                Sps = pss.tile([C, 1], f32, tag="selps")
                nc.tensor.matmul(Sps, ones2_sb[:, :], es[:, :], start=True, stop=True)
                if warm_src is not None:
                    dummy(4)
                invS = selp.tile([C, 1], f32, tag="invS")
                nc.vector.reciprocal(invS, Sps)
                cwps = pss.tile([C, 9], f32, tag="selps")
                for t in range(9):
                    nc.tensor.matmul(cwps[:, t:t + 1],
                                     bankT_sb[:, t * C:(t + 1) * C], es[:, :],
                                     start=True, stop=True)
                cw = selp.tile([C, 9], f32, tag="cw")
                nc.vector.tensor_scalar(cw, cwps, invS[:, :], None, ALU.mult)
                diag = diagp.tile([C, 9 * C], bf16, tag="diag")
                for t in PE_TAPS:
                    nc.vector.tensor_scalar(diag[:, t * C:(t + 1) * C], ident_sb,
                                            cw[:, t:t + 1], None, ALU.mult)
                return diag, cw

            def conv_tile(b, ti, xt, diag, cw):
                r0 = ti * R
                ot = outp.tile([C, R * W], f32, tag="ot")
                for ga in range(0, NPAIR, GRP):
                    grp = list(range(ga, min(ga + GRP, NPAIR)))
                    tsums = {}
                    for j in grp:
                        tap1 = tapp.tile([C, 2 * RS], bf16, tag="tap1")
                        tap4 = tapp.tile([C, 2 * RS], bf16, tag="tap4")
                        # t=1: dh=0,dw=1 -> rows 2j..; t=4: dh=1,dw=1
                        nc.scalar.activation(tap1,
                                             xt[:, LEAD + 2 * j * RS:
                                                LEAD + (2 * j + 2) * RS],
                                             AF.Copy, scale=cw[:, 1:2])
                        nc.scalar.activation(tap4,
                                             xt[:, LEAD + (2 * j + 1) * RS:
                                                LEAD + (2 * j + 3) * RS],
                                             AF.Identity, bias=bias_sb[:, :],
                                             scale=cw[:, 4:5])
                        tsum = tapp.tile([C, 2 * RS], bf16, tag="tsum")
                        nc.vector.tensor_add(tsum, tap1, tap4)
                        tsums[j] = tsum
                    pts = {}
                    for j in grp:
                        pt = psc.tile([C, 2 * RS], f32, tag="pt")
                        pts[j] = pt
                    for t in PE_TAPS:
                        dh, dw = divmod(t, 3)
                        for j in grp:
                            s = LEAD + (2 * j + dh) * RS + dw - 1
                            nc.tensor.matmul(
                                pts[j], diag[:, t * C:(t + 1) * C],
                                xt[:, s:s + 2 * RS],
                                start=(t == PE_TAPS[0]), stop=(t == PE_TAPS[-1]))
                    for j in grp:
                        pt3 = pts[j].rearrange("p (r c) -> p r c", c=RS)
                        ts3 = tsums[j].rearrange("p (r c) -> p r c", c=RS)
                        nc.vector.tensor_add(
                            ot[:, 2 * j * W:(2 * j + 2) * W]
                                .rearrange("p (r c) -> p r c", c=W),
                            pt3[:, :, 0:192], ts3[:, :, 0:192])
                nc.scalar.dma_start(out=out_ext[b][:, r0:r0 + R, :], in_=ot)

            # ---- schedule ----
            partials0 = statp.tile([C, NT], f32, tag="part0")
            xts0 = [load_tile(0, ti, partials0) for ti in range(NT)]
            diag0, cw0 = selector(0, partials0, xts0[NT - 1])
            partials1 = statp.tile([C, NT], f32, tag="part1")
            xts1 = [None] * NT
            for ti in range(NT):
                xts1[ti] = load_tile(1, ti, partials1)
                conv_tile(0, ti, xts0[ti], diag0, cw0)
            diag1, cw1 = selector(1, partials1, None)
            for ti in range(NT):
                conv_tile(1, ti, xts1[ti], diag1, cw1)

    nc.finalize()
    return nc


def kernel(x, w1, b1, w2, b2, weight_bank, bias):
    from concourse.bass_utils import run_bass_kernel_spmd

    x = np.ascontiguousarray(np.asarray(x, dtype=np.float32))
    w1 = np.asarray(w1, dtype=np.float32)
    b1 = np.asarray(b1, dtype=np.float32)
    w2 = np.asarray(w2, dtype=np.float32)
    b2 = np.asarray(b2, dtype=np.float32)
    weight_bank = np.asarray(weight_bank, dtype=np.float32)
    bias = np.asarray(bias, dtype=np.float32)

    if "nc" not in _cache:
        _cache["nc"] = _build()
    nc = _cache["nc"]

    w1T_s = np.ascontiguousarray(w1.T / float(H * W))
    w2T = np.ascontiguousarray(w2.T)
    ones2 = np.ones((2, C), np.float32)
    bankT = np.ascontiguousarray(
        np.transpose(weight_bank.reshape(2, C, 9), (0, 2, 1)).reshape(2, 9 * C))
    ident = np.eye(C, dtype=np.float32)
    common = {
        "w1T": w1T_s, "b1c": np.ascontiguousarray(b1.reshape(32, 1)),
        "w2T": w2T, "b2c": np.ascontiguousarray(b2.reshape(2, 1)),
        "ones2": ones2, "bankT": bankT,
        "biasc": np.ascontiguousarray(bias.reshape(C, 1)), "ident": ident,
    }
    in_maps = [dict(common, x=x[i * BC:(i + 1) * BC]) for i in range(NCORES)]
    res = run_bass_kernel_spmd(nc, in_maps, core_ids=list(range(NCORES)))
    _cache["last_result"] = res
    out = np.concatenate([np.asarray(res.results[i]["out"]) for i in range(NCORES)],
                         axis=0)
    return out


# revision 19
# speedup vs baseline: 1.6693x; 1.6693x over previous
"""Trainium2 Bass kernel for dynamic adaptive-pooling depthwise conv.

Problem: x [16,128,192,192] f32. Per-sample selector head (global mean ->
MLP -> softmax over K=2) mixes a bank of K depthwise 3x3 kernels; then a
per-(sample,channel) 3x3 depthwise conv + bias.

Strategy (8 NeuronCores, data-parallel over batch, 2 samples/core),
single-read design: x is read from HBM exactly once.
  - per 24-row tile: HWDGE f32 load (contiguous) -> GPSIMD casts into a
    RESIDENT zero-padded bf16 SBUF tile, fusing per-channel sums via
    accum_out (mean comes for free).
  - selector head entirely as tiny PE matmuls (no cross-partition ops),
    with PE-warmup dummy matmuls woven through its dependency chain.
  - conv: 7 of 9 taps as PSUM-accumulated diag-stationary matmuls
    (tap-major groups to amortize LDWEIGHTS); the two center-column taps
    (one carrying bias) on ACT (bf16, 4B-aligned for 2x mode); DVE sums
    the two tap maps (bf16 2x) and merges psum+taps into contiguous f32
    staging; stores ride the second HWDGE ring (nc.scalar).

kernel(**inputs) takes FULL inputs, shards batch over 8 cores, returns
FULL output. Self-contained: hardcodes all shapes.
"""
import numpy as np

B, C, H, W = 16, 128, 192, 192
NCORES = 8
BC = B // NCORES          # samples per core
RS = 196                  # padded row stride (192 img + 4 pad cols, even)
LEAD = 2                  # leading pad elems (keeps tap offsets 4B-aligned)
R = 24                    # output rows per conv tile
NT = H // R               # conv tiles per sample (8)
NPAIR = R // 2            # psum row-pairs per conv tile (12)
XT_FLAT = LEAD + (R + 2) * RS + 2
CH = 13                   # max rows per f32 load chunk
GRP = 6                   # row-pairs per psum group (tap-major)
PE_TAPS = [0, 2, 3, 5, 6, 7, 8]   # taps 1 and 4 (center column) on ACT

_cache = {}


def _build():
    from concourse import bacc, mybir
    from concourse.tile import TileContext

    f32 = mybir.dt.float32
    bf16 = mybir.dt.bfloat16
    AF = mybir.ActivationFunctionType
    ALU = mybir.AluOpType
    AX = mybir.AxisListType

    nc = bacc.Bacc()
    x_ext = nc.declare_dram_parameter("x", [BC, C, H, W], f32, isOutput=False)
    out_ext = nc.declare_dram_parameter("out", [BC, C, H, W], f32, isOutput=True)
    w1T_ext = nc.declare_dram_parameter("w1T", [C, 32], f32, isOutput=False)
    b1_ext = nc.declare_dram_parameter("b1c", [32, 1], f32, isOutput=False)
    w2T_ext = nc.declare_dram_parameter("w2T", [32, 2], f32, isOutput=False)
    b2_ext = nc.declare_dram_parameter("b2c", [2, 1], f32, isOutput=False)
    ones2_ext = nc.declare_dram_parameter("ones2", [2, C], f32, isOutput=False)
    bankT_ext = nc.declare_dram_parameter("bankT", [2, 9 * C], f32, isOutput=False)
    bias_ext = nc.declare_dram_parameter("biasc", [C, 1], f32, isOutput=False)
    ident_ext = nc.declare_dram_parameter("ident", [C, C], f32, isOutput=False)

    with TileContext(nc) as tc:
        with (
            tc.tile_pool(name="consts", bufs=1) as consts,
            tc.tile_pool(name="stg", bufs=3) as stgp,
            tc.tile_pool(name="xt", bufs=NT + 1) as xcp,
            tc.tile_pool(name="stat", bufs=2) as statp,
            tc.tile_pool(name="sel", bufs=2) as selp,
            tc.tile_pool(name="diag", bufs=2) as diagp,
            tc.tile_pool(name="tap", bufs=3) as tapp,
            tc.tile_pool(name="outp", bufs=2) as outp,
            tc.tile_pool(name="psc", bufs=6, space="PSUM") as psc,
            tc.tile_pool(name="pss", bufs=2, space="PSUM") as pss,
        ):
            def cload(shape, ext, tag):
                t = consts.tile(shape, f32, tag=tag)
                nc.sync.dma_start(out=t, in_=ext[:, :])
                return t
            w1T_sb = cload([C, 32], w1T_ext, "c_w1T")
            b1_sb = cload([32, 1], b1_ext, "c_b1")
            w2T_sb = cload([32, 2], w2T_ext, "c_w2T")
            b2_sb = cload([2, 1], b2_ext, "c_b2")
            ones2_sb = cload([2, C], ones2_ext, "c_ones2")
            bankT_sb = cload([2, 9 * C], bankT_ext, "c_bankT")
            bias_sb = cload([C, 1], bias_ext, "c_bias")
            ident_sb = cload([C, C], ident_ext, "c_ident")
            identbf = consts.tile([C, C], bf16, tag="c_identbf")
            nc.vector.tensor_copy(identbf, ident_sb)

            def load_tile(b, ti, partials):
                """Load+cast one 24-row tile; fused per-channel sums."""
                r0 = ti * R
                xt = xcp.tile([C, XT_FLAT], bf16, tag="xt")
                xt3 = xt[:, LEAD:LEAD + (R + 2) * RS].rearrange(
                    "p (r c) -> p r c", c=RS)
                nc.gpsimd.memset(xt[:, 0:LEAD], 0.0)
                nc.gpsimd.memset(xt[:, XT_FLAT - 2:XT_FLAT], 0.0)
                nc.gpsimd.memset(xt3[:, :, 192:196], 0.0)
                if ti == 0:
                    nc.gpsimd.memset(xt3[:, 0:1, 0:192], 0.0)
                if ti == NT - 1:
                    nc.gpsimd.memset(xt3[:, R + 1:R + 2, 0:192], 0.0)
                HR = R // 2   # interior rows per chunk (12)
                for half in range(2):
                    if half == 0:
                        qa = r0 - 1 if ti > 0 else r0
                        qb = r0 + HR
                        ha = 1 if ti > 0 else 0
                    else:
                        qa = r0 + HR
                        qb = r0 + 2 * HR + 1 if ti < NT - 1 else r0 + 2 * HR
                        ha = 1 if ti < NT - 1 else 0
                    nrow = qb - qa
                    stg = stgp.tile([C, CH * W], f32, tag="stg")
                    nc.sync.dma_start(out=stg[:, :nrow * W],
                                      in_=x_ext[b][:, qa:qb, :])
                    s3 = stg[:, :nrow * W].rearrange("p (r c) -> p r c", c=W)
                    pc = partials[:, 2 * ti + half:2 * ti + half + 1]
                    if half == 0:
                        if ha:
                            nc.scalar.activation(xt3[:, 0:1, 0:192],
                                                 s3[:, 0:1, :], AF.Copy)
                        nc.scalar.activation(
                            xt3[:, 1:1 + HR, 0:192], s3[:, ha:ha + HR, :],
                            AF.Copy, accum_out=pc)
                    else:
                        nc.scalar.activation(
                            xt3[:, 1 + HR:1 + 2 * HR, 0:192], s3[:, 0:HR, :],
                            AF.Copy, accum_out=pc)
                        if ha:
                            nc.scalar.activation(xt3[:, R + 1:R + 2, 0:192],
                                                 s3[:, HR:HR + 1, :], AF.Copy)
                return xt

            def selector(b, partials, warm_src):
                def dummy(n):
                    for _ in range(n):
                        dps = pss.tile([C, 512], f32, tag="selps")
                        nc.tensor.matmul(dps, identbf[:, :],
                                         warm_src[:, LEAD:LEAD + 512],
                                         start=True, stop=True)
                pooled = statp.tile([C, 1], f32, tag="pooled")
                nc.vector.reduce_sum(pooled, partials, axis=AX.X)
                hA = pss.tile([32, 1], f32, tag="selps")
                nc.tensor.matmul(hA, w1T_sb[:, :], pooled[:, :], start=True, stop=True)
                if warm_src is not None:
                    dummy(4)
                hs = selp.tile([32, 1], f32, tag="hs")
                nc.scalar.activation(hs, hA, AF.Relu, bias=b1_sb[:, :])
                lB = pss.tile([2, 1], f32, tag="selps")
                nc.tensor.matmul(lB, w2T_sb[:, :], hs[:, :], start=True, stop=True)
                if warm_src is not None:
                    dummy(4)
                es = selp.tile([2, 1], f32, tag="es")
                nc.scalar.activation(es, lB, AF.Exp, bias=b2_sb[:, :])
                Sps = pss.tile([C, 1], f32, tag="selps")
                nc.tensor.matmul(Sps, ones2_sb[:, :], es[:, :], start=True, stop=True)
                if warm_src is not None:
                    dummy(4)
                invS = selp.tile([C, 1], f32, tag="invS")
                nc.vector.reciprocal(invS, Sps)
                cwps = pss.tile([C, 9], f32, tag="selps")
                for t in range(9):
                    nc.tensor.matmul(cwps[:, t:t + 1],
                                     bankT_sb[:, t * C:(t + 1) * C], es[:, :],
                                     start=True, stop=True)
                cw = selp.tile([C, 9], f32, tag="cw")
                nc.vector.tensor_scalar(cw, cwps, invS[:, :], None, ALU.mult)
                diag = diagp.tile([C, 9 * C], bf16, tag="diag")
                for t in PE_TAPS:
                    nc.vector.tensor_scalar(diag[:, t * C:(t + 1) * C], ident_sb,
                                            cw[:, t:t + 1], None, ALU.mult)
                return diag, cw

            def conv_tile(b, ti, xt, diag, cw):
                r0 = ti * R
                ot = outp.tile([C, R * W], f32, tag="ot")
                for ga in range(0, NPAIR, GRP):
                    grp = list(range(ga, min(ga + GRP, NPAIR)))
                    glen = 2 * len(grp) * RS
                    # group-batched center-column taps: t=1 on ACT, t=4
                    # fused with the sum on DVE (scalar_tensor_tensor)
                    c1 = LEAD + 2 * ga * RS
                    c4 = LEAD + (2 * ga + 1) * RS
                    tap1g = tapp.tile([C, 2 * GRP * RS], bf16, tag="tap1g")
                    nc.scalar.activation(tap1g[:, :glen], xt[:, c1:c1 + glen],
                                         AF.Copy, scale=cw[:, 1:2])
                    tsumg = tapp.tile([C, 2 * GRP * RS], bf16, tag="tsumg")
                    nc.vector.scalar_tensor_tensor(
                        tsumg[:, :glen], xt[:, c4:c4 + glen], cw[:, 4:5],
                        tap1g[:, :glen], ALU.mult, ALU.add)
                    ts3 = tsumg.rearrange("p (r c) -> p r c", c=RS)
                    pts = {}
                    for j in grp:
                        pt = psc.tile([C, 2 * RS], f32, tag="pt")
                        pts[j] = pt
                    for t in PE_TAPS:
                        dh, dw = divmod(t, 3)
                        for j in grp:
                            s = LEAD + (2 * j + dh) * RS + dw - 1
                            nc.tensor.matmul(
                                pts[j], diag[:, t * C:(t + 1) * C],
                                xt[:, s:s + 2 * RS],
                                start=(t == PE_TAPS[0]), stop=(t == PE_TAPS[-1]))
                    for j in grp:
                        pt3 = pts[j].rearrange("p (r c) -> p r c", c=RS)
                        g = 2 * (j - ga)
                        nc.vector.scalar_tensor_tensor(
                            ot[:, 2 * j * W:(2 * j + 2) * W]
                                .rearrange("p (r c) -> p r c", c=W),
                            pt3[:, :, 0:192], bias_sb[:, :],
                            ts3[:, g:g + 2, 0:192], ALU.add, ALU.add)
                nc.scalar.dma_start(out=out_ext[b][:, r0:r0 + R, :], in_=ot)

            # ---- schedule ----
            partials0 = statp.tile([C, 2 * NT], f32, tag="part0")
            xts0 = [load_tile(0, ti, partials0) for ti in range(NT)]
            diag0, cw0 = selector(0, partials0, xts0[NT - 1])
            partials1 = statp.tile([C, 2 * NT], f32, tag="part1")
            xts1 = [None] * NT
            for ti in range(NT):
                xts1[ti] = load_tile(1, ti, partials1)
                conv_tile(0, ti, xts0[ti], diag0, cw0)
            diag1, cw1 = selector(1, partials1, None)
            for ti in range(NT):
                conv_tile(1, ti, xts1[ti], diag1, cw1)

    nc.finalize()
    return nc


def kernel(x, w1, b1, w2, b2, weight_bank, bias):
    from concourse.bass_utils import run_bass_kernel_spmd

    x = np.ascontiguousarray(np.asarray(x, dtype=np.float32))
    w1 = np.asarray(w1, dtype=np.float32)
    b1 = np.asarray(b1, dtype=np.float32)
    w2 = np.asarray(w2, dtype=np.float32)
    b2 = np.asarray(b2, dtype=np.float32)
    weight_bank = np.asarray(weight_bank, dtype=np.float32)
    bias = np.asarray(bias, dtype=np.float32)

    if "nc" not in _cache:
        _cache["nc"] = _build()
    nc = _cache["nc"]

    w1T_s = np.ascontiguousarray(w1.T / float(H * W))
    w2T = np.ascontiguousarray(w2.T)
    ones2 = np.ones((2, C), np.float32)
    bankT = np.ascontiguousarray(
        np.transpose(weight_bank.reshape(2, C, 9), (0, 2, 1)).reshape(2, 9 * C))
    ident = np.eye(C, dtype=np.float32)
    common = {
        "w1T": w1T_s, "b1c": np.ascontiguousarray(b1.reshape(32, 1)),
        "w2T": w2T, "b2c": np.ascontiguousarray(b2.reshape(2, 1)),
        "ones2": ones2, "bankT": bankT,
        "biasc": np.ascontiguousarray(bias.reshape(C, 1)), "ident": ident,
    }
    in_maps = [dict(common, x=x[i * BC:(i + 1) * BC]) for i in range(NCORES)]
    res = run_bass_kernel_spmd(nc, in_maps, core_ids=list(range(NCORES)))
    _cache["last_result"] = res
    out = np.concatenate([np.asarray(res.results[i]["out"]) for i in range(NCORES)],
                         axis=0)
    return out
